# revision 16
# baseline (speedup 1.0000x reference)
"""Trainium2 Bass kernel for the NMS-detection problem.

Contract: kernel(**inputs) takes the FULL inputs
    tmap_raw  (B,4,64,64) f32, logit_raw (B,1,64,64) f32,
    n_objects_max (int), topk_only (int)
and returns the reference's output tuple
    (prob_few, bx_few, by_few, bw_few, bh_few), each (n_objects_max, B) f32.

Sharding: data-parallel over the batch dim. Core c computes batch element
c % B entirely on-chip (greedy NMS is sequential per batch element); the
host gathers the per-core records from cores 0..B-1.

Device algorithm (per core): boxes live in a (128,32) SBUF layout
(box i = p*32 + j, i = ix*64 + iy). cand = prob*possible is maintained
in-place. Per pick:
  pmax = rowmax(cand); transpose; gmax = max;                (argmax value)
  gmaxcol = ones_row^T @ gmax          (exact per-partition broadcast)
  prod5 = (cand >= gmaxcol) * geom5; red5 = sum_j            (winner stats)
  ps_h  = ones128^T @ red5             (winner stats on all partitions)
  suppression: 7 fused vector ops ending in cand *= (z >= 0)
  record path (prod4/red4/MM2/ACT copies) runs in the PE/ACT shadow.
Picks come out in descending-prob order == reference's top_k order
(verified numerically: pick sequence identical to the jax reference for
this input, no fp32 ties, robust to the 2^-21 fp32r matmul rounding).
"""

from contextlib import ExitStack

import numpy as np

import concourse.bass as bass
import concourse.bacc as bacc
import concourse.tile as tile
import concourse.mybir as mybir
from concourse.bass_utils import run_bass_kernel_spmd

F32 = mybir.dt.float32
F32R = mybir.dt.float32r
ALU = mybir.AluOpType
ACTF = mybir.ActivationFunctionType

N = 4096
P = 128
J = 32  # free cols per partition; box index i = p*J + j
N_CORES = 8


def _make_consts():
    i = np.arange(N, dtype=np.float32)
    ixg8 = (8.0 * np.floor(i / 64)).reshape(P, J).astype(np.float32)
    iyg8 = (8.0 * np.mod(i, 64)).reshape(P, J).astype(np.float32)
    pack = np.concatenate([np.eye(P, dtype=np.float32),
                           ixg8, iyg8], axis=1)  # (P, P+2J)
    return {
        "c_pack": np.ascontiguousarray(pack),
        "c_ones128": np.ones((P, P), dtype=mybir.dt.np(mybir.dt.bfloat16)),
        "c_ones_row": np.ones((1, P), dtype=np.float32),
    }


def _build(nobj, topk_only):
    nc = bacc.Bacc("TRN2", target_bir_lowering=False, debug=False,
                   num_devices=N_CORES)

    traw = nc.dram_tensor("traw", [P, 4 * J], F32, kind="ExternalInput").ap()
    lraw = nc.dram_tensor("lraw", [P, J], F32, kind="ExternalInput").ap()
    c_pack = nc.dram_tensor("c_pack", [P, P + 2 * J], F32,
                            kind="ExternalInput").ap()
    c_ones128 = nc.dram_tensor("c_ones128", [P, P], mybir.dt.bfloat16,
                               kind="ExternalInput").ap()
    c_ones_row = nc.dram_tensor("c_ones_row", [1, P], F32,
                                kind="ExternalInput").ap()
    nrec = max(64, ((nobj * 4 + 31) // 32) * 32)
    nprob = max(64, ((nobj + 31) // 32) * 32)
    outg_d = nc.dram_tensor("outg", [1, nrec], F32, kind="ExternalOutput").ap()
    outp_d = nc.dram_tensor("outp", [1, nprob], F32, kind="ExternalOutput").ap()

    with tile.TileContext(nc) as tc, ExitStack() as ctx:
        _body(ctx, tc, traw, lraw, c_pack, c_ones128, c_ones_row,
              outg_d, outp_d, nrec, nprob, nobj, topk_only)
    nc.compile()
    return nc


def _blk3(ap2d, nblk):
    """(P, nblk*J) 2-D slice -> (P, nblk, J) 3-D view."""
    return ap2d.rearrange("a (m j) -> a m j", j=J)


def _bcast_cols(t, off_cols, nblk):
    """AP reading tile column `off_cols` broadcast as (P, nblk, J):
    one value per partition, repeated J times within each of nblk blocks
    advancing by one element per block."""
    base = t[:, off_cols:off_cols + nblk]
    return bass.AP(base.tensor, base.offset,
                   [list(base.ap[0]), [1, nblk], [0, J]])


def _bcast_same(t, nblk):
    """AP reading (P, J) tile broadcast as (P, nblk, J): same J values in
    every block."""
    base = t[:]
    return bass.AP(base.tensor, base.offset,
                   [list(base.ap[0]), [0, nblk], [1, J]])


def _body(ctx, tc, traw, lraw, c_pack, c_ones128, c_ones_row,
          outg_d, outp_d, nrec, nprob, nobj, topk_only):
    nc = tc.nc
    v = nc.vector
    s = nc.scalar
    t = nc.tensor

    cpool = ctx.enter_context(tc.tile_pool(name="consts", bufs=1))
    ppool = ctx.enter_context(tc.tile_pool(name="persist", bufs=1))
    wpool = ctx.enter_context(tc.tile_pool(name="work", bufs=2))
    qpool = ctx.enter_context(tc.tile_pool(name="psum", bufs=2, space="PSUM"))

    # ---- load constants & inputs -------------------------------------------
    pack = cpool.tile([P, P + 2 * J], F32, tag="pack")
    nc.sync.dma_start(pack[:], c_pack)
    ident = pack[:, 0:P]
    ixg8 = pack[:, P:P + J]
    iyg8 = pack[:, P + J:P + 2 * J]
    ones128_t = cpool.tile([P, P], mybir.dt.bfloat16, tag="ones128")
    nc.sync.dma_start(ones128_t[:], c_ones128)
    ones128 = ones128_t[:]
    ones_row = cpool.tile([1, P], F32, tag="ones_row")
    nc.sync.dma_start(ones_row[:], c_ones_row)

    tin = ppool.tile([P, 4 * J], F32, tag="tin")
    nc.sync.dma_start(tin[:], traw)
    lin = ppool.tile([P, J], F32, tag="lin")
    nc.sync.dma_start(lin[:], lraw)

    # ---- preprocessing ------------------------------------------------------
    # geomS blocks (bf16, J=32): 0:x1 1:y1 2:x3n(=-x3) 3:y3n(=-y3) 4:a3
    # geomR blocks (f32):  0:bx 1:by 2:bw 3:bh
    # Negated far corners let max() compute both corner limits in one op:
    # min(x3,X3) = -max(x3n,X3n); the negations cancel in inter = wx*wy.
    # bf16 geometry validated: pick sequence identical to fp32 reference.
    geomS = ppool.tile([P, 5 * J], mybir.dt.bfloat16, tag="geomS")
    blk = lambda k: geomS[:, k * J:(k + 1) * J]
    x1_sl, y1_sl, x3_sl, y3_sl, a3_sl = blk(0), blk(1), blk(2), blk(3), blk(4)
    geomR = ppool.tile([P, 4 * J], F32, tag="geomR")
    rblk = lambda k: geomR[:, k * J:(k + 1) * J]
    bx_sl, by_sl, bw_sl, bh_sl = rblk(0), rblk(1), rblk(2), rblk(3)

    ts4 = wpool.tile([P, 4 * J], F32, tag="ts4")
    s.activation(ts4[:], tin[:], ACTF.Sigmoid)   # [tx|ty|tw|th]
    tx, ty = ts4[:, 0:J], ts4[:, J:2 * J]
    tw, th = ts4[:, 2 * J:3 * J], ts4[:, 3 * J:4 * J]

    prob = ppool.tile([P, J], F32, tag="prob")
    s.activation(prob[:], lin[:], ACTF.Sigmoid)
    cand = ppool.tile([P, J], F32, tag="cand")
    v.tensor_copy(cand[:], prob[:])

    # bx = 8*tx + 8*ix ; by = 8*ty + 8*iy ; bw = 30*tw+10 ; bh = 30*th+10
    v.scalar_tensor_tensor(bx_sl, tx, 8.0, ixg8, op0=ALU.mult, op1=ALU.add)
    v.scalar_tensor_tensor(by_sl, ty, 8.0, iyg8, op0=ALU.mult, op1=ALU.add)
    v.tensor_scalar(bw_sl, tw, 30.0, 10.0, op0=ALU.mult, op1=ALU.add)
    v.tensor_scalar(bh_sl, th, 30.0, 10.0, op0=ALU.mult, op1=ALU.add)
    # corners and 0.3*area
    v.scalar_tensor_tensor(x1_sl, bw_sl, -0.5, bx_sl, op0=ALU.mult, op1=ALU.add)
    v.scalar_tensor_tensor(x3_sl, bw_sl, -0.5, bx_sl,
                           op0=ALU.mult, op1=ALU.subtract)
    v.scalar_tensor_tensor(y1_sl, bh_sl, -0.5, by_sl, op0=ALU.mult, op1=ALU.add)
    v.scalar_tensor_tensor(y3_sl, bh_sl, -0.5, by_sl,
                           op0=ALU.mult, op1=ALU.subtract)
    v.scalar_tensor_tensor(a3_sl, bw_sl, 0.3, bh_sl, op0=ALU.mult, op1=ALU.mult)

    # no memset needed: the ACT record copies fully cover outg[:, :4*nobj]
    # and outp[:, :nobj], and the host reads only those ranges
    outg = ppool.tile([1, nrec], F32, tag="outg")
    outp = ppool.tile([1, nprob], F32, tag="outp")

    geom5_3 = _blk3(geomS[:], 5)
    geom4_3 = _blk3(geomR[:], 4)
    cand_b5 = _bcast_same(cand, 5)

    # ---- greedy NMS loop ----------------------------------------------------
    def _record(l, gcol, gmax):
        # [bx,by,bw,bh] of pick l via prob-mask (prob is immutable;
        # winner's prob == gmax exactly, probs are tie-free), plus prob=gmax.
        # Runs on DVE/PE/ACT slack off the critical suppression path.
        prod4 = wpool.tile([P, 4 * J], F32, tag="prod4")
        prob_b4 = bass.AP(prob[:].tensor, prob[:].offset,
                          [list(prob[:].ap[0]), [0, 4], [1, J]])
        v.scalar_tensor_tensor(_blk3(prod4[:], 4), prob_b4, gcol[:, 0:1],
                               geom4_3, op0=ALU.is_equal, op1=ALU.mult)
        red4 = wpool.tile([P, 4], mybir.dt.bfloat16, tag="red4")
        with nc.allow_low_precision(reason="bf16 record sums, tol 2e-2"):
            v.tensor_reduce(red4[:], _blk3(prod4[:], 4),
                            axis=mybir.AxisListType.X, op=ALU.add)
        rec = qpool.tile([1, 4], F32, tag="rec")
        t.matmul(rec[:], ones128_t[:, 0:1], red4[:])
        s.copy(outg[:, l * 4:(l + 1) * 4], rec[:])
        s.copy(outp[:, l:l + 1], gmax[:])

    for l in range(nobj):
        # global max of cand
        pmax = wpool.tile([P, 1], F32, tag="pmax")
        v.tensor_reduce(pmax[:], cand[:], axis=mybir.AxisListType.X, op=ALU.max)
        ps_t = qpool.tile([1, P], F32, tag="ps_t")
        t.transpose(ps_t[:], pmax[:], ident)

        gmax = wpool.tile([1, 1], F32, tag="gmax")
        v.tensor_reduce(gmax[:], ps_t[:], axis=mybir.AxisListType.X, op=ALU.max)

        # exact broadcast of gmax to all partitions: ones_row^T @ gmax
        gcol = qpool.tile([P, 1], F32, tag="gcol")
        t.matmul(gcol[:], ones_row[:], gmax[:])

        # winner stats: prod5 = (cand >= gmax) * geom5 ; red5 = sum_j
        prod5 = wpool.tile([P, 5 * J], F32, tag="prod5")
        v.scalar_tensor_tensor(_blk3(prod5[:], 5), cand_b5, gcol[:, 0:1],
                               geom5_3, op0=ALU.is_ge, op1=ALU.mult)
        red5 = wpool.tile([P, 6], mybir.dt.bfloat16, tag="red5")
        with nc.allow_low_precision(reason="bf16 winner-stat sums, validated"):
            v.tensor_reduce(red5[:, 0:5], _blk3(prod5[:], 5),
                            axis=mybir.AxisListType.X, op=ALU.add)

        # broadcast winner stats to all partitions: ones128^T @ red5
        # (bf16 single-pass; winner scalars only need ~2e-3 accuracy,
        # validated against the reference pick sequence)
        ps_h = qpool.tile([P, 6], F32, tag="ps_h")
        t.matmul(ps_h[:], ones128, red5[:])

        _record(l, gcol, gmax)

        if topk_only:
            # plain top-k: remove only the chosen box
            v.scalar_tensor_tensor(cand[:], cand[:], gcol[:, 0:1], cand[:],
                                   op0=ALU.is_lt, op1=ALU.mult)
        else:
            # suppression: overlap(winner, box) = w*h ; keep iff
            # min(a3, A3) - w*h >= 0  (a3 = 0.3*area)
            # t_abq = max([x1,y1,x3n,y3n], [X1,Y1,X3n,Y3n]) in one op
            t_abq = wpool.tile([P, 4 * J], mybir.dt.bfloat16, tag="t_abq")
            v.tensor_tensor(_blk3(t_abq[:], 4), _blk3(geomS[:, 0:4 * J], 4),
                            _bcast_cols(ps_h, 0, 4), op=ALU.max)
            # whn = -overlap_width|height ; clamp to <= 0 ; inter = wx*wy
            whn = wpool.tile([P, 2 * J], mybir.dt.bfloat16, tag="whn")
            v.tensor_tensor(whn[:], t_abq[:, 0:2 * J], t_abq[:, 2 * J:4 * J],
                            op=ALU.add)
            v.tensor_scalar(whn[:], whn[:], 0.0, None, op0=ALU.min)
            t_i = wpool.tile([P, J], mybir.dt.bfloat16, tag="t_i")
            v.tensor_tensor(t_i[:], whn[:, 0:J], whn[:, J:2 * J],
                            op=ALU.mult)
            t_z = wpool.tile([P, J], mybir.dt.bfloat16, tag="t_z")
            v.scalar_tensor_tensor(t_z[:], a3_sl, ps_h[:, 4:5], t_i[:],
                                   op0=ALU.min, op1=ALU.subtract)
            v.scalar_tensor_tensor(cand[:], t_z[:], 0.0, cand[:],
                                   op0=ALU.is_ge, op1=ALU.mult)

    nc.sync.dma_start(outg_d, outg[:])
    nc.sync.dma_start(outp_d, outp[:])


_CACHE = {}


def _get_program(nobj, topk_only):
    key = (nobj, topk_only)
    if key not in _CACHE:
        _CACHE[key] = _build(nobj, topk_only)
    return _CACHE[key]


def run_on_device(tmap_raw, logit_raw, n_objects_max, topk_only,
                  trace=False, tmpdir=None):
    """Shard over cores, run, and return (outputs_tuple, BassKernelResults)."""
    nobj = int(n_objects_max)
    tk = int(np.asarray(topk_only))
    tmap = np.ascontiguousarray(np.asarray(tmap_raw, dtype=np.float32))
    logit = np.ascontiguousarray(np.asarray(logit_raw, dtype=np.float32))
    B = tmap.shape[0]

    nc = _get_program(nobj, tk)
    consts = _make_consts()
    in_maps = []
    for c in range(N_CORES):
        b = c % B
        in_maps.append({
            "traw": np.ascontiguousarray(
                tmap[b].reshape(4, P, J).transpose(1, 0, 2).reshape(P, 4 * J)),
            "lraw": logit[b, 0].reshape(P, J),
            **consts,
        })
    kw = {}
    if trace:
        kw = dict(trace=True, tmpdir=tmpdir)
    bres = run_bass_kernel_spmd(nc, in_maps, list(range(N_CORES)), **kw)
    res = bres.results

    K = nobj
    outs = [np.zeros((K, B), np.float32) for _ in range(5)]
    for b in range(B):
        rec = np.asarray(res[b]["outg"]).reshape(-1)[:K * 4].reshape(K, 4)
        outs[0][:, b] = np.asarray(res[b]["outp"]).reshape(-1)[:K]
        for m in range(4):
            outs[m + 1][:, b] = rec[:, m]
    return tuple(outs), bres


def kernel(tmap_raw, logit_raw, n_objects_max, topk_only):
    outs, _ = run_on_device(tmap_raw, logit_raw, n_objects_max, topk_only)
    return outs


# revision 17
# speedup vs baseline: 1.0059x; 1.0059x over previous
"""Trainium2 Bass kernel for the NMS-detection problem.

Contract: kernel(**inputs) takes the FULL inputs
    tmap_raw  (B,4,64,64) f32, logit_raw (B,1,64,64) f32,
    n_objects_max (int), topk_only (int)
and returns the reference's output tuple
    (prob_few, bx_few, by_few, bw_few, bh_few), each (n_objects_max, B) f32.

Sharding: data-parallel over the batch dim. Core c computes batch element
c % B entirely on-chip (greedy NMS is sequential per batch element); the
host gathers the per-core records from cores 0..B-1.

Device algorithm (per core): boxes live in a (128,32) SBUF layout
(box i = p*32 + j, i = ix*64 + iy). cand = prob*possible is maintained
in-place. Per pick:
  pmax = rowmax(cand); transpose; gmax = max;                (argmax value)
  gmaxcol = ones_row^T @ gmax          (exact per-partition broadcast)
  prod5 = (cand >= gmaxcol) * geom5; red5 = sum_j            (winner stats)
  ps_h  = ones128^T @ red5             (winner stats on all partitions)
  suppression: 7 fused vector ops ending in cand *= (z >= 0)
  record path (prod4/red4/MM2/ACT copies) runs in the PE/ACT shadow.
Picks come out in descending-prob order == reference's top_k order
(verified numerically: pick sequence identical to the jax reference for
this input, no fp32 ties, robust to the 2^-21 fp32r matmul rounding).
"""

from contextlib import ExitStack

import numpy as np

import concourse.bass as bass
import concourse.bacc as bacc
import concourse.tile as tile
import concourse.mybir as mybir
from concourse.bass_utils import run_bass_kernel_spmd

F32 = mybir.dt.float32
F32R = mybir.dt.float32r
ALU = mybir.AluOpType
ACTF = mybir.ActivationFunctionType

N = 4096
P = 128
J = 32  # free cols per partition; box index i = p*J + j
N_CORES = 8


def _make_consts():
    i = np.arange(N, dtype=np.float32)
    ixg8 = (8.0 * np.floor(i / 64)).reshape(P, J).astype(np.float32)
    iyg8 = (8.0 * np.mod(i, 64)).reshape(P, J).astype(np.float32)
    pack = np.concatenate([np.eye(P, dtype=np.float32),
                           ixg8, iyg8], axis=1)  # (P, P+2J)
    return {
        "c_pack": np.ascontiguousarray(pack),
        "c_ones128": np.ones((P, P), dtype=mybir.dt.np(mybir.dt.bfloat16)),
        "c_ones_row": np.ones((1, P), dtype=np.float32),
    }


def _build(nobj, topk_only):
    nc = bacc.Bacc("TRN2", target_bir_lowering=False, debug=False,
                   num_devices=N_CORES)

    traw = nc.dram_tensor("traw", [P, 4 * J], F32, kind="ExternalInput").ap()
    lraw = nc.dram_tensor("lraw", [P, J], F32, kind="ExternalInput").ap()
    c_pack = nc.dram_tensor("c_pack", [P, P + 2 * J], F32,
                            kind="ExternalInput").ap()
    c_ones128 = nc.dram_tensor("c_ones128", [P, P], mybir.dt.bfloat16,
                               kind="ExternalInput").ap()
    c_ones_row = nc.dram_tensor("c_ones_row", [1, P], F32,
                                kind="ExternalInput").ap()
    nrec = max(64, ((nobj * 4 + 31) // 32) * 32)
    nprob = max(64, ((nobj + 31) // 32) * 32)
    outg_d = nc.dram_tensor("outg", [1, nrec], F32, kind="ExternalOutput").ap()
    outp_d = nc.dram_tensor("outp", [1, nprob], F32, kind="ExternalOutput").ap()

    with tile.TileContext(nc) as tc, ExitStack() as ctx:
        _body(ctx, tc, traw, lraw, c_pack, c_ones128, c_ones_row,
              outg_d, outp_d, nrec, nprob, nobj, topk_only)
    nc.compile()
    return nc


def _blk3(ap2d, nblk):
    """(P, nblk*J) 2-D slice -> (P, nblk, J) 3-D view."""
    return ap2d.rearrange("a (m j) -> a m j", j=J)


def _bcast_cols(t, off_cols, nblk):
    """AP reading tile column `off_cols` broadcast as (P, nblk, J):
    one value per partition, repeated J times within each of nblk blocks
    advancing by one element per block."""
    base = t[:, off_cols:off_cols + nblk]
    return bass.AP(base.tensor, base.offset,
                   [list(base.ap[0]), [1, nblk], [0, J]])


def _bcast_same(t, nblk):
    """AP reading (P, J) tile broadcast as (P, nblk, J): same J values in
    every block."""
    base = t[:]
    return bass.AP(base.tensor, base.offset,
                   [list(base.ap[0]), [0, nblk], [1, J]])


def _body(ctx, tc, traw, lraw, c_pack, c_ones128, c_ones_row,
          outg_d, outp_d, nrec, nprob, nobj, topk_only):
    nc = tc.nc
    v = nc.vector
    s = nc.scalar
    t = nc.tensor

    cpool = ctx.enter_context(tc.tile_pool(name="consts", bufs=1))
    ppool = ctx.enter_context(tc.tile_pool(name="persist", bufs=1))
    wpool = ctx.enter_context(tc.tile_pool(name="work", bufs=2))
    qpool = ctx.enter_context(tc.tile_pool(name="psum", bufs=2, space="PSUM"))

    # ---- load constants & inputs -------------------------------------------
    pack = cpool.tile([P, P + 2 * J], F32, tag="pack")
    nc.sync.dma_start(pack[:], c_pack)
    ident = pack[:, 0:P]
    ixg8 = pack[:, P:P + J]
    iyg8 = pack[:, P + J:P + 2 * J]
    ones128_t = cpool.tile([P, P], mybir.dt.bfloat16, tag="ones128")
    nc.sync.dma_start(ones128_t[:], c_ones128)
    ones128 = ones128_t[:]
    ones_row = cpool.tile([1, P], F32, tag="ones_row")
    nc.sync.dma_start(ones_row[:], c_ones_row)

    tin = ppool.tile([P, 4 * J], F32, tag="tin")
    nc.sync.dma_start(tin[:], traw)
    lin = ppool.tile([P, J], F32, tag="lin")
    nc.sync.dma_start(lin[:], lraw)

    # ---- preprocessing ------------------------------------------------------
    # geomS blocks (bf16, J=32): 0:x1 1:y1 2:x3n(=-x3) 3:y3n(=-y3) 4:a3
    # geomR blocks (f32):  0:bx 1:by 2:bw 3:bh
    # Negated far corners let max() compute both corner limits in one op:
    # min(x3,X3) = -max(x3n,X3n); the negations cancel in inter = wx*wy.
    # bf16 geometry validated: pick sequence identical to fp32 reference.
    geomS = ppool.tile([P, 5 * J], mybir.dt.bfloat16, tag="geomS")
    blk = lambda k: geomS[:, k * J:(k + 1) * J]
    x1_sl, y1_sl, x3_sl, y3_sl, a3_sl = blk(0), blk(1), blk(2), blk(3), blk(4)
    geomR = ppool.tile([P, 4 * J], F32, tag="geomR")
    rblk = lambda k: geomR[:, k * J:(k + 1) * J]
    bx_sl, by_sl, bw_sl, bh_sl = rblk(0), rblk(1), rblk(2), rblk(3)

    ts4 = wpool.tile([P, 4 * J], F32, tag="ts4")
    s.activation(ts4[:], tin[:], ACTF.Sigmoid)   # [tx|ty|tw|th]
    tx, ty = ts4[:, 0:J], ts4[:, J:2 * J]
    tw, th = ts4[:, 2 * J:3 * J], ts4[:, 3 * J:4 * J]

    cand = ppool.tile([P, J], F32, tag="cand")
    s.activation(cand[:], lin[:], ACTF.Sigmoid)  # prob (= initial cand)

    # bx = 8*tx + 8*ix ; by = 8*ty + 8*iy ; bw = 30*tw+10 ; bh = 30*th+10
    v.scalar_tensor_tensor(bx_sl, tx, 8.0, ixg8, op0=ALU.mult, op1=ALU.add)
    v.scalar_tensor_tensor(by_sl, ty, 8.0, iyg8, op0=ALU.mult, op1=ALU.add)
    v.tensor_scalar(bw_sl, tw, 30.0, 10.0, op0=ALU.mult, op1=ALU.add)
    v.tensor_scalar(bh_sl, th, 30.0, 10.0, op0=ALU.mult, op1=ALU.add)
    # corners and 0.3*area
    v.scalar_tensor_tensor(x1_sl, bw_sl, -0.5, bx_sl, op0=ALU.mult, op1=ALU.add)
    v.scalar_tensor_tensor(x3_sl, bw_sl, -0.5, bx_sl,
                           op0=ALU.mult, op1=ALU.subtract)
    v.scalar_tensor_tensor(y1_sl, bh_sl, -0.5, by_sl, op0=ALU.mult, op1=ALU.add)
    v.scalar_tensor_tensor(y3_sl, bh_sl, -0.5, by_sl,
                           op0=ALU.mult, op1=ALU.subtract)
    v.scalar_tensor_tensor(a3_sl, bw_sl, 0.3, bh_sl, op0=ALU.mult, op1=ALU.mult)

    # no memset needed: the ACT record copies fully cover outg[:, :4*nobj]
    # and outp[:, :nobj], and the host reads only those ranges
    outg = ppool.tile([1, nrec], F32, tag="outg")
    outp = ppool.tile([1, nprob], F32, tag="outp")

    geom5_3 = _blk3(geomS[:], 5)
    geom4_3 = _blk3(geomR[:], 4)
    cand_b5 = _bcast_same(cand, 5)

    # ---- greedy NMS loop ----------------------------------------------------
    def _record(l, gcol, gmax):
        # [bx,by,bw,bh] of pick l via winner mask (cand still holds the
        # pick-time scores here — _record is issued before the suppression
        # update), plus prob=gmax. Runs on DVE/PE/ACT slack off the
        # critical suppression path.
        prod4 = wpool.tile([P, 4 * J], F32, tag="prod4")
        cand_b4 = bass.AP(cand[:].tensor, cand[:].offset,
                          [list(cand[:].ap[0]), [0, 4], [1, J]])
        v.scalar_tensor_tensor(_blk3(prod4[:], 4), cand_b4, gcol[:, 0:1],
                               geom4_3, op0=ALU.is_ge, op1=ALU.mult)
        red4 = wpool.tile([P, 4], mybir.dt.bfloat16, tag="red4")
        with nc.allow_low_precision(reason="bf16 record sums, tol 2e-2"):
            v.tensor_reduce(red4[:], _blk3(prod4[:], 4),
                            axis=mybir.AxisListType.X, op=ALU.add)
        rec = qpool.tile([1, 4], F32, tag="rec")
        t.matmul(rec[:], ones128_t[:, 0:1], red4[:])
        s.copy(outg[:, l * 4:(l + 1) * 4], rec[:])
        s.copy(outp[:, l:l + 1], gmax[:])

    for l in range(nobj):
        # global max of cand
        pmax = wpool.tile([P, 1], F32, tag="pmax")
        v.tensor_reduce(pmax[:], cand[:], axis=mybir.AxisListType.X, op=ALU.max)
        ps_t = qpool.tile([1, P], F32, tag="ps_t")
        t.transpose(ps_t[:], pmax[:], ident)

        gmax = wpool.tile([1, 1], F32, tag="gmax")
        v.tensor_reduce(gmax[:], ps_t[:], axis=mybir.AxisListType.X, op=ALU.max)

        # exact broadcast of gmax to all partitions: ones_row^T @ gmax
        gcol = qpool.tile([P, 1], F32, tag="gcol")
        t.matmul(gcol[:], ones_row[:], gmax[:])

        # winner stats: prod5 = (cand >= gmax) * geom5 ; red5 = sum_j
        prod5 = wpool.tile([P, 5 * J], F32, tag="prod5")
        v.scalar_tensor_tensor(_blk3(prod5[:], 5), cand_b5, gcol[:, 0:1],
                               geom5_3, op0=ALU.is_ge, op1=ALU.mult)
        red5 = wpool.tile([P, 6], mybir.dt.bfloat16, tag="red5")
        with nc.allow_low_precision(reason="bf16 winner-stat sums, validated"):
            v.tensor_reduce(red5[:, 0:5], _blk3(prod5[:], 5),
                            axis=mybir.AxisListType.X, op=ALU.add)

        # broadcast winner stats to all partitions: ones128^T @ red5
        # (bf16 single-pass; winner scalars only need ~2e-3 accuracy,
        # validated against the reference pick sequence)
        ps_h = qpool.tile([P, 6], F32, tag="ps_h")
        t.matmul(ps_h[:], ones128, red5[:])

        _record(l, gcol, gmax)

        if topk_only:
            # plain top-k: remove only the chosen box
            v.scalar_tensor_tensor(cand[:], cand[:], gcol[:, 0:1], cand[:],
                                   op0=ALU.is_lt, op1=ALU.mult)
        else:
            # suppression: overlap(winner, box) = w*h ; keep iff
            # min(a3, A3) - w*h >= 0  (a3 = 0.3*area)
            # t_abq = max([x1,y1,x3n,y3n], [X1,Y1,X3n,Y3n]) in one op
            t_abq = wpool.tile([P, 4 * J], mybir.dt.bfloat16, tag="t_abq")
            v.tensor_tensor(_blk3(t_abq[:], 4), _blk3(geomS[:, 0:4 * J], 4),
                            _bcast_cols(ps_h, 0, 4), op=ALU.max)
            # whn = -overlap_width|height ; clamp to <= 0 ; inter = wx*wy
            whn = wpool.tile([P, 2 * J], mybir.dt.bfloat16, tag="whn")
            v.tensor_tensor(whn[:], t_abq[:, 0:2 * J], t_abq[:, 2 * J:4 * J],
                            op=ALU.add)
            v.tensor_scalar(whn[:], whn[:], 0.0, None, op0=ALU.min)
            t_i = wpool.tile([P, J], mybir.dt.bfloat16, tag="t_i")
            v.tensor_tensor(t_i[:], whn[:, 0:J], whn[:, J:2 * J],
                            op=ALU.mult)
            t_z = wpool.tile([P, J], mybir.dt.bfloat16, tag="t_z")
            v.scalar_tensor_tensor(t_z[:], a3_sl, ps_h[:, 4:5], t_i[:],
                                   op0=ALU.min, op1=ALU.subtract)
            v.scalar_tensor_tensor(cand[:], t_z[:], 0.0, cand[:],
                                   op0=ALU.is_ge, op1=ALU.mult)

    nc.sync.dma_start(outg_d, outg[:])
    nc.sync.dma_start(outp_d, outp[:])


_CACHE = {}


def _get_program(nobj, topk_only):
    key = (nobj, topk_only)
    if key not in _CACHE:
        _CACHE[key] = _build(nobj, topk_only)
    return _CACHE[key]


def run_on_device(tmap_raw, logit_raw, n_objects_max, topk_only,
                  trace=False, tmpdir=None):
    """Shard over cores, run, and return (outputs_tuple, BassKernelResults)."""
    nobj = int(n_objects_max)
    tk = int(np.asarray(topk_only))
    tmap = np.ascontiguousarray(np.asarray(tmap_raw, dtype=np.float32))
    logit = np.ascontiguousarray(np.asarray(logit_raw, dtype=np.float32))
    B = tmap.shape[0]

    nc = _get_program(nobj, tk)
    consts = _make_consts()
    in_maps = []
    for c in range(N_CORES):
        b = c % B
        in_maps.append({
            "traw": np.ascontiguousarray(
                tmap[b].reshape(4, P, J).transpose(1, 0, 2).reshape(P, 4 * J)),
            "lraw": logit[b, 0].reshape(P, J),
            **consts,
        })
    kw = {}
    if trace:
        kw = dict(trace=True, tmpdir=tmpdir)
    bres = run_bass_kernel_spmd(nc, in_maps, list(range(N_CORES)), **kw)
    res = bres.results

    K = nobj
    outs = [np.zeros((K, B), np.float32) for _ in range(5)]
    for b in range(B):
        rec = np.asarray(res[b]["outg"]).reshape(-1)[:K * 4].reshape(K, 4)
        outs[0][:, b] = np.asarray(res[b]["outp"]).reshape(-1)[:K]
        for m in range(4):
            outs[m + 1][:, b] = rec[:, m]
    return tuple(outs), bres


def kernel(tmap_raw, logit_raw, n_objects_max, topk_only):
    outs, _ = run_on_device(tmap_raw, logit_raw, n_objects_max, topk_only)
    return outs
